# revision 27
# baseline (speedup 1.0000x reference)
"""Binarized VGG-style CNN (CIFAR, batch 256) on 8 TRN2 NeuronCores.

Data-parallel: batch 256 -> 8 x 32. One Bass program, per-core input maps.

Math: for every conv layer 1..6 the network only consumes sign(BN(...)),
and BN is monotone (gamma>0 here), so each layer reduces to a threshold
test on the integer conv sum. Activations are stored in the {0,1}
"b-encoding" (a = 2b - 1, zero-pad border = 0.5): with binary weights
w in {+-1}, sum w*a = 2*sum w*b - W where W = sum w is a per-cout
constant, so the threshold becomes  b_next = [psum_b >= (W - be)/2].
That makes the activation a single is_ge op on the vector engine (or an
equivalent steep-sigmoid on the scalar engine, so the two engines share
the work). conv2..7 run in fp8 with DoubleRow pairing (fp32 PSUM);
conv1 (real input) runs as one bf16 K=81 matmul (triple-split input). The final BN1d + log_softmax runs on
the host (it is 256x10 and off the device critical path).
"""

import numpy as np

import concourse.bass as bass
import concourse.bacc as bacc
import concourse.tile as tile
import concourse.mybir as mybir
from concourse.bass_utils import run_bass_kernel_spmd

F32 = mybir.dt.float32
BF16 = mybir.dt.bfloat16
FP16 = mybir.dt.float16
FP8 = mybir.dt.float8e4
NP_FP8 = mybir.dt.np(FP8)
NP_BF16 = mybir.dt.np(BF16)
NP_FP16 = mybir.dt.np(FP16)

N_CORES = 8
B = 32  # images per core
EPS = 1e-5
PAD = 0.5      # b-encoding of a zero activation
SIG_SCALE = float(2 ** 24)

ALU = mybir.AluOpType
ACTF = mybir.ActivationFunctionType
PM = mybir.MatmulPerfMode

# layer configs for conv2..conv6
CONV_CFG = {
    2: dict(IG=1, OG=1),
    3: dict(IG=1, OG=2),
    4: dict(IG=2, OG=2),
    5: dict(IG=2, OG=4),
    6: dict(IG=4, OG=4),
}

# plane geometry: images packed side-by-side along width, shared separator
# cols (PAD), pad rows top/bottom, 16-elem guard at both ends.
# SZ must be a multiple of 16 (DoubleRow cg-pair stride).
PLANE = {
    1: dict(Wp=1072, W=32, H=32, stride=33, SZ=34 * 1072 + 32),   # P1 / L2 in
    2: dict(Wp=560, W=16, H=16, stride=17, SZ=18 * 560 + 32),     # P2,P3
    3: dict(Wp=289, W=8, H=8, stride=9, SZ=2928),                 # P4,P5
}
SZ1, SZ2, SZ3 = PLANE[1]["SZ"], PLANE[2]["SZ"], PLANE[3]["SZ"]
assert SZ2 % 16 == 0 and SZ3 % 16 == 0

_CACHE = {}


def _pl_chunks(Wp, Hval):
    """512-chunks over valid rows 1..Hval; returns (abs_lin, n)."""
    total = Hval * Wp
    out, o = [], 0
    while o < total:
        n = min(512, total - o)
        out.append((Wp + o, n))
        o += n
    return out


def _ap(base, off, dims):
    return bass.AP(tensor=base.tensor, offset=base.offset + off, ap=[base.ap[0]] + dims)


def _build_v3(dump=False):
    nc = bacc.Bacc("TRN2", target_bir_lowering=False, debug=False)

    # im2col input, bf16 triple split stacked along K (fp32-exact):
    # rows 0-26 = bf16(x), 27-53 = bf16(r1), 54-80 = bf16(r2); w1 tiled 3x.
    # layout [half, quad, 81, 4img*16row*32col]: wave 0 (image rows 1-16)
    # lands first so conv2's lower band can start while wave 1 streams in.
    xim_d = nc.dram_tensor("xim", [2, B // 8, 81, 4096], BF16, kind="ExternalInput")
    w1_d = nc.dram_tensor("w1s", [81, 128], BF16, kind="ExternalInput")
    w_d = {}
    for l in (2, 3):
        w_d[l] = nc.dram_tensor(f"w{l}p", [128, 3, 3, 128 * CONV_CFG[l]["OG"]], FP8,
                                kind="ExternalInput")
    for l in (4, 5, 6):
        c = CONV_CFG[l]
        w_d[l] = nc.dram_tensor(
            f"w{l}s", [128, c["IG"], 9, c["OG"], 128], FP8, kind="ExternalInput"
        )
    # all is_ge thresholds and sigmoid biases packed into one tensor:
    # cols 0..13 = th for l=1..6 (og-major), cols 14..27 = sg
    ths_d = nc.dram_tensor("ths", [128, 28], F32, kind="ExternalInput")
    w7_d = nc.dram_tensor("w7p", [128, 4, 4, 2, 2, 16], FP8, kind="ExternalInput")
    out_d = nc.dram_tensor("out", [10, B], F32, kind="ExternalOutput")

    with tile.TileContext(nc) as tc:
        with (
            tc.tile_pool(name="wpool", bufs=1) as wpool,
            tc.tile_pool(name="apool", bufs=1) as apool,
            tc.tile_pool(name="xim", bufs=2) as xim,
            tc.tile_pool(name="xim2", bufs=4) as xim2,
            tc.tile_pool(name="tpool", bufs=4) as tpool,
            tc.tile_pool(name="spool", bufs=2) as spool,
            tc.tile_pool(name="psum", bufs=6, space="PSUM") as pp,
            tc.tile_pool(name="psum7", bufs=1, space="PSUM") as pp7,
            tc.tile_pool(name="scrpool", bufs=1) as scrpool,
        ):
            # ---- warm-up stationary (no DMA dependency) ----
            wz = wpool.tile([128, 128], BF16, tag="wz")
            nc.vector.memset(wz[:], 0.0)

            w1_t = wpool.tile([81, 128], BF16, tag="w1")
            nc.gpsimd.dma_start(w1_t[:], w1_d[:])
            ths_t = wpool.tile([128, 28], F32, tag="ths")
            nc.gpsimd.dma_start(ths_t[:], ths_d[:])
            _off = {1: 0, 2: 1, 3: 2, 4: 4, 5: 6, 6: 10}
            th_t = {l: ths_t[:, o:o + (1 if l == 1 else CONV_CFG[l]["OG"])]
                    for l, o in _off.items()}
            sg_t = {l: ths_t[:, 14 + o:14 + o + (1 if l == 1 else CONV_CFG[l]["OG"])]
                    for l, o in _off.items()}

            w_t = {}

            def load_weight(l, eng):
                # staged per layer so the bytes never compete with the
                # conv1 im2col feed for DMA queue bandwidth
                if l in (2, 3):
                    w_t[l] = wpool.tile([128, 3, 3, 128 * CONV_CFG[l]["OG"]], FP8,
                                        tag=f"w{l}", name=f"w{l}t")
                else:
                    c = CONV_CFG[l]
                    w_t[l] = wpool.tile([128, c["IG"], 9, c["OG"], 128], FP8,
                                        tag=f"w{l}", name=f"w{l}t")
                eng.dma_start(w_t[l][:], w_d[l][:])

            w7_t = wpool.tile([128, 4, 4, 2, 2, 16], FP8, tag="w7")

            # activation planes ({0,1} b-encoding, pads PAD)
            P1 = apool.tile([128, SZ1], FP8, tag="P1")
            P2 = apool.tile([128, SZ2], FP8, tag="P2")
            P3 = apool.tile([128, 2, SZ2], FP8, tag="P3")
            P4 = apool.tile([128, 2, SZ3], FP8, tag="P4")
            P5 = apool.tile([128, 4, SZ3], FP8, tag="P5")
            # [128, g(4), dy(4), dx(4), i(32)]
            buf6 = apool.tile([128, 4, 4, 4, B], FP8, tag="buf6")

            def pad_sep(Pt, goff, pl, tail=False):
                """Separator cols (incl left pad col), all rows; optionally
                unused tail cols (for planes whose signs overwrite them)."""
                Wp, H, st = pl["Wp"], pl["H"], pl["stride"]
                base = Pt[:]
                nc.gpsimd.memset(
                    _ap(base, goff + 16, [[Wp, H + 2], [st, B + 1]]), PAD)
                used = st * B + 1
                if tail and Wp > used:
                    nc.gpsimd.memset(
                        _ap(base, goff + 16 + used, [[Wp, H + 2], [1, Wp - used]]),
                        PAD)

            def pad_static(Pt, goff, pl, tail=True):
                """Top/bottom pad rows, head/tail guards, unused tail cols —
                regions no sign/pool ever writes."""
                Wp, H, st = pl["Wp"], pl["H"], pl["stride"]
                base = Pt[:]
                nc.gpsimd.memset(_ap(base, goff + 16, [[1, Wp]]), PAD)
                nc.gpsimd.memset(
                    _ap(base, goff + 16 + (H + 1) * Wp, [[1, Wp]]), PAD)
                used = st * B + 1
                if tail and Wp > used:
                    nc.gpsimd.memset(
                        _ap(base, goff + 16 + used, [[Wp, H + 2], [1, Wp - used]]),
                        PAD)
                nc.gpsimd.memset(_ap(base, goff, [[1, 16]]), PAD)
                tail_off = goff + 16 + (H + 2) * Wp
                ntail = pl["SZ"] - (16 + (H + 2) * Wp)
                nc.gpsimd.memset(_ap(base, tail_off, [[1, ntail]]), PAD)

            # ---- PE warm-up: burn the cold-frequency window during the
            # initial DMA wait (no data dependency; result discarded) ----
            for _ in range(10):
                psd = pp.tile([128, 512], F32, tag="ps", name="psd")
                nc.tensor.matmul(psd[:, :128], wz[:], wz[:], start=True, stop=True)

            # ---- sign emission: alternate vector(is_ge) / scalar(sigmoid) ----
            def emit_sign(dst, src, l, og, use_v):
                if use_v:
                    nc.vector.tensor_scalar(dst, src, th_t[l][:, og:og + 1], None,
                                            op0=ALU.is_ge)
                else:
                    nc.scalar.activation(dst, src, ACTF.Sigmoid,
                                         bias=sg_t[l][:, og:og + 1],
                                         scale=SIG_SCALE)


            # ---- conv1 wave 0 (image rows 1-16), 8-image octs (8KB lines) ----
            # octs ride the two HW-DGE queues (sync/scalar) only: the gpsimd
            # software queue's descriptor pump stalls whenever the gpsimd
            # engine runs memsets.
            wave_engs = [nc.scalar, nc.sync, nc.gpsimd, nc.scalar]

            def conv1_half(h, q, im):
                for j in range(8):
                    i = 8 * q + j
                    ps = pp.tile([128, 16, 32], F32, tag="ps")
                    nc.tensor.matmul(ps[:], w1_t[:], im[:, j],
                                     start=True, stop=True)
                    dst = _ap(P1[:], 16 + (1 + 16 * h) * 1072 + 33 * i + 1,
                              [[1072, 16], [1, 32]])
                    emit_sign(dst, ps[:], 1, 0, (2 * i + h) % 2 == 0)

            for q in range(B // 8):
                im = xim.tile([81, 8, 16, 32], BF16, tag="imh")
                wave_engs[q].dma_start(
                    im[:], xim_d[0, q].rearrange("k (j r w) -> k j r w", j=8, w=32))
                conv1_half(0, q, im)
            # w2 is needed by band A (~25us): put it on sync's queue
            # ahead of sync's wave-1 octs
            load_weight(2, nc.sync)
            # wave 1 DMA issues go out now (gpsimd/sync only — scalar's
            # stream is about to fill with band-A sigmoids). One buf per
            # oct: NO slot reuse (readers are emitted much later).
            w2_engs = [nc.sync, nc.gpsimd, nc.sync, nc.gpsimd]
            im2_tiles = []
            for q in range(B // 8):
                im = xim2.tile([81, 8, 16, 32], BF16, tag="imh2")
                w2_engs[q].dma_start(
                    im[:], xim_d[1, q].rearrange("k (j r w) -> k j r w", j=8, w=32))
                im2_tiles.append(im)
            load_weight(3, nc.sync)
            pad_static(P1, 0, PLANE[1])
            pad_sep(P1, 0, PLANE[1])

            # ---- dy-paired layer (IG=1): L2 (pool, banded) and L3 ----
            def mm_dy_pairs(Pin, wt, og, o, n, Wp, ps):
                k, last = 0, 4
                osl = slice(og * 128, (og + 1) * 128)
                for dx in range(3):
                    rhs = _ap(Pin[:], 16 + o - Wp + dx - 1, [[Wp, 2], [1, n]])
                    nc.tensor.matmul(ps[:], wt[:, dx, 0:2, osl], rhs,
                                     start=(k == 0), stop=(k == last),
                                     perf_mode=PM.DoubleRow)
                    k += 1
                rhs = _ap(Pin[:], 16 + o + Wp - 1, [[1, 2], [1, n]])
                nc.tensor.matmul(ps[:], wt[:, 0:2, 2, osl], rhs,
                                 start=(k == 0), stop=(k == last),
                                 perf_mode=PM.DoubleRow)
                k += 1
                rhs = _ap(Pin[:], 16 + o + Wp + 1, [[1, n]])
                nc.tensor.matmul(ps[:], wt[:, 2, 2, osl], rhs,
                                 start=(k == 0), stop=(k == last))
                k += 1

            def mm_cg_pairs(Pin, wt, og, o, n, Wp, SZg, IG, ps):
                k, last = 0, IG // 2 * 9 - 1
                for pr in range(IG // 2):
                    for dy in range(3):
                        for dx in range(3):
                            rhs = _ap(Pin[:], 2 * pr * SZg + 16 + o + (dy - 1) * Wp + dx - 1,
                                      [[SZg, 2], [1, n]])
                            nc.tensor.matmul(
                                ps[:], wt[:, 2 * pr: 2 * pr + 2, 3 * dy + dx, og, :],
                                rhs, start=(k == 0), stop=(k == last),
                                perf_mode=PM.DoubleRow)
                            k += 1

            def pool_row(scr, loc_row, Wp_in, st_in, W_half, dst_ap, tag):
                # 2x2 maxpool of scratch rows loc_row, loc_row+1 -> dst_ap
                m1 = tpool.tile([128, B, W_half], FP8, tag=f"m1{tag}")
                m2 = tpool.tile([128, B, W_half], FP8, tag=f"m2{tag}")
                for j, m in ((0, m1), (1, m2)):
                    off = (loc_row + j) * Wp_in + 1
                    nc.vector.tensor_max(
                        m[:],
                        _ap(scr[:], off, [[st_in, B], [2, W_half]]),
                        _ap(scr[:], off + 1, [[st_in, B], [2, W_half]]),
                    )
                nc.vector.tensor_max(dst_ap, m1[:], m2[:])

            # L2 in two bands: band A (out rows 1-14) only needs wave-0
            # P1 rows, so it overlaps the wave-1 DMA; band B (rows 15-32)
            # runs after conv1 wave 1.
            sk = 0  # global sign-engine alternator for conv layers

            def l2_band(row0, nrows, R0, R1):
                nonlocal sk
                scr2 = scrpool.tile([128, 18 * 1072], FP8, tag="scr")
                band0 = row0 * 1072
                total = nrows * 1072
                o = 0
                while o < total:
                    n = min(512, total - o)
                    ps = pp.tile([128, 512], F32, tag="ps")
                    mm_dy_pairs(P1, w_t[2], 0, band0 + o, n, 1072, ps[:, :n])
                    emit_sign(scr2[:, o: o + n], ps[:, :n], 2, 0, sk % 2 == 0)
                    sk += 1
                    o += n
                for R in range(R0, R1):
                    loc = (2 * R - 1) - row0
                    pool_row(scr2, loc, 1072, 33, 16,
                             _ap(P2[:], 16 + R * 560 + 1, [[17, 32], [1, 16]]), "a")

            l2_band(1, 14, 1, 8)
            # ---- conv1 wave 1 (image rows 17-32) ----
            for q in range(B // 8):
                conv1_half(1, q, im2_tiles[q])
            pad_static(P2, 0, PLANE[2])
            pad_sep(P2, 0, PLANE[2])
            for og in range(2):
                pad_static(P3, og * SZ2, PLANE[2], tail=False)
            l2_band(15, 18, 8, 17)

            load_weight(4, nc.sync)
            for og in range(2):
                pad_static(P4, og * SZ3, PLANE[3])
                pad_sep(P4, og * SZ3, PLANE[3])
            # L3
            for og in range(2):
                for (o, n) in _pl_chunks(560, 16):
                    ps = pp.tile([128, 512], F32, tag="ps")
                    mm_dy_pairs(P2, w_t[3], og, o, n, 560, ps[:, :n])
                    emit_sign(P3[:, og, 16 + o: 16 + o + n], ps[:, :n], 3, og,
                              sk % 2 == 0)
                    sk += 1
                pad_sep(P3, og * SZ2, PLANE[2], tail=True)

            load_weight(5, nc.sync)
            for og in range(4):
                pad_static(P5, og * SZ3, PLANE[3], tail=False)
            # L4 (cg pairs, pool)
            for og in range(2):
                scr4 = scrpool.tile([128, 16 * 1072], FP8, tag="scr")
                for (o, n) in _pl_chunks(560, 16):
                    ps = pp.tile([128, 512], F32, tag="ps")
                    mm_cg_pairs(P3, w_t[4], og, o, n, 560, SZ2, 2, ps[:, :n])
                    emit_sign(scr4[:, o - 560: o - 560 + n], ps[:, :n], 4, og,
                              sk % 2 == 0)
                    sk += 1
                for R in range(1, 9):
                    pool_row(scr4, 2 * (R - 1), 560, 17, 8,
                             _ap(P4[:], og * SZ3 + 16 + R * 289 + 1, [[9, 32], [1, 8]]),
                             "b")

            load_weight(6, nc.sync)
            nc.sync.dma_start(w7_t[:], w7_d[:])
            # L5
            for og in range(4):
                for (o, n) in _pl_chunks(289, 8):
                    ps = pp.tile([128, 512], F32, tag="ps")
                    mm_cg_pairs(P4, w_t[5], og, o, n, 289, SZ3, 2, ps[:, :n])
                    emit_sign(P5[:, og, 16 + o: 16 + o + n], ps[:, :n], 5, og,
                              sk % 2 == 0)
                    sk += 1
                pad_sep(P5, og * SZ3, PLANE[3], tail=True)

            # L6 (cg pairs x2, pool) with conv7 interleaved per og
            ps7 = pp7.tile([16, B], F32, tag="ps7")
            for og in range(4):
                scr6 = scrpool.tile([128, 16 * 1072], FP8, tag="scr")
                for (o, n) in _pl_chunks(289, 8):
                    ps = pp.tile([128, 512], F32, tag="ps")
                    mm_cg_pairs(P5, w_t[6], og, o, n, 289, SZ3, 4, ps[:, :n])
                    emit_sign(scr6[:, o - 289: o - 289 + n], ps[:, :n], 6, og,
                              sk % 2 == 0)
                    sk += 1
                for R in range(1, 5):
                    m1 = tpool.tile([128, B, 4], FP8, tag="m1c")
                    m2 = tpool.tile([128, B, 4], FP8, tag="m2c")
                    for j, m in ((0, m1), (1, m2)):
                        off = (2 * (R - 1) + j) * 289 + 1
                        nc.vector.tensor_max(
                            m[:],
                            _ap(scr6[:], off, [[9, B], [2, 4]]),
                            _ap(scr6[:], off + 1, [[9, B], [2, 4]]),
                        )
                    dst = _ap(buf6[:], og * 512 + (R - 1) * 128, [[32, 4], [1, B]])
                    nc.vector.tensor_max(dst,
                                         _ap(m1[:], 0, [[1, 4], [4, B]]),
                                         _ap(m2[:], 0, [[1, 4], [4, B]]))
                # conv7: stationary = w7 [128,2,16] (dy-pairs), moving = buf6
                for dx in range(4):
                    for p in range(2):
                        rhs = _ap(buf6[:], og * 512 + (2 * p) * 128 + dx * 32,
                                  [[128, 2], [1, B]])
                        nc.tensor.matmul(ps7[:], w7_t[:, og, dx, p], rhs,
                                         start=(og == 0 and dx == 0 and p == 0),
                                         stop=(og == 3 and dx == 3 and p == 1),
                                         perf_mode=PM.DoubleRow)

            # ---- copy conv7 psum out (BN + log_softmax run on the host) ----
            res = spool.tile([10, B], F32, tag="res")
            nc.scalar.copy(res[:], ps7[0:10, :])
            nc.sync.dma_start(out_d[:], res[:])

            if dump:
                for nm, bt in [("dbgP1", P1), ("dbgP2", P2), ("dbgP3", P3),
                               ("dbgP4", P4), ("dbgP5", P5), ("dbg6", buf6)]:
                    dd = nc.dram_tensor(nm, list(bt.shape), FP8, kind="ExternalOutput")
                    nc.sync.dma_start(dd[:], bt[:])

    nc.compile()
    return nc


_THOFF = {1: 0, 2: 1, 3: 2, 4: 4, 5: 6, 6: 10}


def _prep_consts(inp):
    """Host-side weight preprocessing -> (device input dict, host aux)."""
    out = {}
    ths = np.zeros((128, 28), np.float64)
    # device im2col partition order is k = dy*9 + c*3 + dx
    out["w1s"] = np.ascontiguousarray(np.tile(
        np.sign(inp["w1"]).transpose(2, 1, 3, 0).reshape(27, 128), (3, 1)
    )).astype(NP_BF16)
    for l in (4, 5, 6):
        c = CONV_CFG[l]
        IG, OG = c["IG"], c["OG"]
        ws = np.sign(inp[f"w{l}"]).astype(np.float32)  # [cout, cin, 3, 3]
        ws = ws.transpose(1, 2, 3, 0).reshape(IG, 128, 9, OG, 128)
        out[f"w{l}s"] = np.ascontiguousarray(ws.transpose(1, 0, 2, 3, 4)).astype(NP_FP8)
    for l in (2, 3):
        # dy-pair layout: [128(cin), dx, dy, cout]
        ws = np.sign(inp[f"w{l}"]).astype(np.float32)
        out[f"w{l}p"] = np.ascontiguousarray(ws.transpose(1, 3, 2, 0)).astype(NP_FP8)
    # thresholds: layer l>=2 in b-encoding: psum_b >= (W - be)/2
    for l in range(1, 7):
        g = inp[f"bn{l}_g"].astype(np.float64)
        bb = inp[f"bn{l}_b"].astype(np.float64)
        m = inp[f"bn{l}_m"].astype(np.float64)
        v = inp[f"bn{l}_v"].astype(np.float64)
        s = g / np.sqrt(v + EPS)
        t = m - bb / s
        be = inp[f"b{l}"].astype(np.float64) - t
        if l == 1:
            tp = -be
        else:
            W = np.sign(inp[f"w{l}"]).astype(np.float64).sum(axis=(1, 2, 3))
            tp = (W - be) / 2.0
        C = tp.shape[0]
        OG = C // 128
        tp = tp.reshape(OG, 128).T if OG > 1 else tp.reshape(128, 1)
        ths[:, _THOFF[l]:_THOFF[l] + tp.shape[1]] = tp
        ths[:, 14 + _THOFF[l]:14 + _THOFF[l] + tp.shape[1]] = -SIG_SCALE * tp
    # conv7: [128cin, g, dx, pair, dy-in-pair, 16(10+pad)]
    ws7 = np.sign(inp["w7"]).astype(np.float32)  # [10, 512, 4, 4]
    w7p = np.zeros((128, 4, 4, 2, 2, 16), np.float32)
    s7 = ws7.transpose(1, 2, 3, 0).reshape(4, 128, 4, 4, 10)  # [g, c, dy, dx, co]
    for p in range(2):
        for q in range(2):
            w7p[:, :, :, p, q, :10] = s7[:, :, 2 * p + q, :, :].transpose(1, 0, 2, 3)
    out["w7p"] = np.ascontiguousarray(w7p).astype(NP_FP8)
    W7 = np.sign(inp["w7"]).astype(np.float64).sum(axis=(1, 2, 3))  # [10]
    sf = inp["bnf_g"].astype(np.float64) / np.sqrt(inp["bnf_v"].astype(np.float64) + EPS)
    df = (inp["b7"].astype(np.float64) - inp["bnf_m"].astype(np.float64)) * sf + inp[
        "bnf_b"
    ].astype(np.float64)
    out["ths"] = np.ascontiguousarray(ths).astype(np.float32)
    aux = dict(W7=W7, sf=sf, df=df)
    return out, aux


def _prep_x_im2col(x):
    """[b,3,32,32] -> [b,27,2,1024] zero-padded im2col (bf16 hi/lo split),
    k = dy*9 + c*3 + dx."""
    b = x.shape[0]
    xp = np.zeros((b, 3, 34, 34), np.float32)
    xp[:, :, 1:33, 1:33] = x
    xim = np.empty((b, 27, 32, 32), np.float32)
    for dy in range(3):
        for c in range(3):
            for dx in range(3):
                xim[:, dy * 9 + c * 3 + dx] = xp[:, c, dy: dy + 32, dx: dx + 32]
    xim = xim.reshape(b, 27, 1024)
    t0 = xim.astype(NP_BF16)
    r1 = xim - t0.astype(np.float32)
    t1 = r1.astype(NP_BF16)
    t2 = (r1 - t1.astype(np.float32)).astype(NP_BF16)
    full = np.concatenate([t0, t1, t2], axis=1)          # [b, 81, 1024]
    # [half, oct, 81, (8 img, 16 rows, 32 cols)]
    v = full.reshape(b // 8, 8, 81, 2, 16 * 32)
    v = v.transpose(3, 0, 2, 1, 4)                       # [2, o, 81, 8, 512]
    return np.ascontiguousarray(v.reshape(2, b // 8, 81, 4096))


def make_in_maps(inputs):
    consts, aux = _prep_consts(inputs)
    x = np.asarray(inputs["x"], dtype=np.float32)
    in_maps = []
    for c in range(N_CORES):
        m = dict(consts)
        m["xim"] = _prep_x_im2col(x[c * B: (c + 1) * B])
        in_maps.append(m)
    return in_maps, aux


def finish_host(res_list, aux):
    """[10,B] per core psum_b -> full [256,10] log_softmax output."""
    psb = np.concatenate([np.asarray(r["out"], np.float64) for r in res_list],
                         axis=1)  # [10, 256]
    true = 2.0 * psb - aux["W7"][:, None]
    z = (true * aux["sf"][:, None] + aux["df"][:, None]).T  # [256, 10]
    zm = z - z.max(axis=1, keepdims=True)
    out = zm - np.log(np.exp(zm).sum(axis=1, keepdims=True))
    return out.astype(np.float32)


def kernel(**inputs) -> np.ndarray:
    inputs = {k: np.asarray(v) for k, v in inputs.items()}
    if "nc" not in _CACHE:
        _CACHE["nc"] = _build_v3()
    nc = _CACHE["nc"]
    in_maps, aux = make_in_maps(inputs)
    res = run_bass_kernel_spmd(nc, in_maps, list(range(N_CORES)))
    return finish_host(res.results, aux)


# revision 29
# speedup vs baseline: 1.0159x; 1.0159x over previous
"""Binarized VGG-style CNN (CIFAR, batch 256) on 8 TRN2 NeuronCores.

Data-parallel: batch 256 -> 8 x 32. One Bass program, per-core input maps.

Math: for every conv layer 1..6 the network only consumes sign(BN(...)),
and BN is monotone (gamma>0 here), so each layer reduces to a threshold
test on the integer conv sum. Activations are stored in the {0,1}
"b-encoding" (a = 2b - 1, zero-pad border = 0.5): with binary weights
w in {+-1}, sum w*a = 2*sum w*b - W where W = sum w is a per-cout
constant, so the threshold becomes  b_next = [psum_b >= (W - be)/2].
That makes the activation a single is_ge op on the vector engine (or an
equivalent steep-sigmoid on the scalar engine, so the two engines share
the work). conv2..7 run in fp8 with DoubleRow pairing (fp32 PSUM);
conv1 (real input) runs as one bf16 K=81 matmul (triple-split input). The final BN1d + log_softmax runs on
the host (it is 256x10 and off the device critical path).
"""

import numpy as np

import concourse.bass as bass
import concourse.bacc as bacc
import concourse.tile as tile
import concourse.mybir as mybir
from concourse.bass_utils import run_bass_kernel_spmd

F32 = mybir.dt.float32
BF16 = mybir.dt.bfloat16
FP16 = mybir.dt.float16
FP8 = mybir.dt.float8e4
NP_FP8 = mybir.dt.np(FP8)
NP_BF16 = mybir.dt.np(BF16)
NP_FP16 = mybir.dt.np(FP16)

N_CORES = 8
B = 32  # images per core
EPS = 1e-5
PAD = 0.5      # b-encoding of a zero activation
SIG_SCALE = float(2 ** 24)

ALU = mybir.AluOpType
ACTF = mybir.ActivationFunctionType
PM = mybir.MatmulPerfMode

# layer configs for conv2..conv6
CONV_CFG = {
    2: dict(IG=1, OG=1),
    3: dict(IG=1, OG=2),
    4: dict(IG=2, OG=2),
    5: dict(IG=2, OG=4),
    6: dict(IG=4, OG=4),
}

# plane geometry: images packed side-by-side along width, shared separator
# cols (PAD), pad rows top/bottom, 16-elem guard at both ends.
# SZ must be a multiple of 16 (DoubleRow cg-pair stride).
PLANE = {
    1: dict(Wp=1072, W=32, H=32, stride=33, SZ=34 * 1072 + 32),   # P1 / L2 in
    2: dict(Wp=560, W=16, H=16, stride=17, SZ=18 * 560 + 32),     # P2,P3
    3: dict(Wp=289, W=8, H=8, stride=9, SZ=2928),                 # P4,P5
}
SZ1, SZ2, SZ3 = PLANE[1]["SZ"], PLANE[2]["SZ"], PLANE[3]["SZ"]
assert SZ2 % 16 == 0 and SZ3 % 16 == 0

_CACHE = {}


def _pl_chunks(Wp, Hval):
    """512-chunks over valid rows 1..Hval; returns (abs_lin, n)."""
    total = Hval * Wp
    out, o = [], 0
    while o < total:
        n = min(512, total - o)
        out.append((Wp + o, n))
        o += n
    return out


def _ap(base, off, dims):
    return bass.AP(tensor=base.tensor, offset=base.offset + off, ap=[base.ap[0]] + dims)


def _build_v3(dump=False):
    nc = bacc.Bacc("TRN2", target_bir_lowering=False, debug=False)

    # im2col input, bf16 triple split stacked along K (fp32-exact):
    # rows 0-26 = bf16(x), 27-53 = bf16(r1), 54-80 = bf16(r2); w1 tiled 3x.
    # layout [half, quad, 81, 4img*16row*32col]: wave 0 (image rows 1-16)
    # lands first so conv2's lower band can start while wave 1 streams in.
    xim_d = nc.dram_tensor("xim", [2, B // 8, 81, 4096], BF16, kind="ExternalInput")
    w1_d = nc.dram_tensor("w1s", [81, 128], BF16, kind="ExternalInput")
    w_d = {}
    for l in (2, 3):
        w_d[l] = nc.dram_tensor(f"w{l}p", [128, 3, 3, 128 * CONV_CFG[l]["OG"]], FP8,
                                kind="ExternalInput")
    for l in (4, 5, 6):
        c = CONV_CFG[l]
        w_d[l] = nc.dram_tensor(
            f"w{l}s", [128, c["IG"], 9, c["OG"], 128], FP8, kind="ExternalInput"
        )
    # all is_ge thresholds and sigmoid biases packed into one tensor:
    # cols 0..13 = th for l=1..6 (og-major), cols 14..27 = sg
    ths_d = nc.dram_tensor("ths", [128, 28], F32, kind="ExternalInput")
    w7_d = nc.dram_tensor("w7p", [128, 4, 4, 2, 2, 16], FP8, kind="ExternalInput")
    out_d = nc.dram_tensor("out", [10, B], F32, kind="ExternalOutput")

    with tile.TileContext(nc) as tc:
        with (
            tc.tile_pool(name="wpool", bufs=1) as wpool,
            tc.tile_pool(name="apool", bufs=1) as apool,
            tc.tile_pool(name="xim", bufs=2) as xim,
            tc.tile_pool(name="xim2", bufs=4) as xim2,
            tc.tile_pool(name="tpool", bufs=4) as tpool,
            tc.tile_pool(name="spool", bufs=2) as spool,
            tc.tile_pool(name="psum", bufs=6, space="PSUM") as pp,
            tc.tile_pool(name="psum7", bufs=1, space="PSUM") as pp7,
            tc.tile_pool(name="scrpool", bufs=1) as scrpool,
        ):
            # ---- warm-up stationary (no DMA dependency) ----
            wz = wpool.tile([128, 128], BF16, tag="wz")
            nc.vector.memset(wz[:], 0.0)

            w1_t = wpool.tile([81, 128], BF16, tag="w1")
            nc.gpsimd.dma_start(w1_t[:], w1_d[:])
            ths_t = wpool.tile([128, 28], F32, tag="ths")
            nc.gpsimd.dma_start(ths_t[:], ths_d[:])
            _off = {1: 0, 2: 1, 3: 2, 4: 4, 5: 6, 6: 10}
            th_t = {l: ths_t[:, o:o + (1 if l == 1 else CONV_CFG[l]["OG"])]
                    for l, o in _off.items()}
            sg_t = {l: ths_t[:, 14 + o:14 + o + (1 if l == 1 else CONV_CFG[l]["OG"])]
                    for l, o in _off.items()}

            w_t = {}

            def load_weight(l, eng):
                # staged per layer so the bytes never compete with the
                # conv1 im2col feed for DMA queue bandwidth
                if l in (2, 3):
                    w_t[l] = wpool.tile([128, 3, 3, 128 * CONV_CFG[l]["OG"]], FP8,
                                        tag=f"w{l}", name=f"w{l}t")
                else:
                    c = CONV_CFG[l]
                    w_t[l] = wpool.tile([128, c["IG"], 9, c["OG"], 128], FP8,
                                        tag=f"w{l}", name=f"w{l}t")
                eng.dma_start(w_t[l][:], w_d[l][:])

            w7_t = wpool.tile([128, 4, 4, 2, 2, 16], FP8, tag="w7")

            # activation planes ({0,1} b-encoding, pads PAD)
            P1 = apool.tile([128, SZ1], FP8, tag="P1")
            P2 = apool.tile([128, SZ2], FP8, tag="P2")
            P3 = apool.tile([128, 2, SZ2], FP8, tag="P3")
            P4 = apool.tile([128, 2, SZ3], FP8, tag="P4")
            P5 = apool.tile([128, 4, SZ3], FP8, tag="P5")
            # [128, g(4), dy(4), dx(4), i(32)]
            buf6 = apool.tile([128, 4, 4, 4, B], FP8, tag="buf6")

            def pad_sep(Pt, goff, pl, tail=False):
                """Separator cols (incl left pad col), all rows; optionally
                unused tail cols (for planes whose signs overwrite them)."""
                Wp, H, st = pl["Wp"], pl["H"], pl["stride"]
                base = Pt[:]
                nc.gpsimd.memset(
                    _ap(base, goff + 16, [[Wp, H + 2], [st, B + 1]]), PAD)
                used = st * B + 1
                if tail and Wp > used:
                    nc.gpsimd.memset(
                        _ap(base, goff + 16 + used, [[Wp, H + 2], [1, Wp - used]]),
                        PAD)

            def pad_static(Pt, goff, pl, tail=True):
                """Top/bottom pad rows, head/tail guards, unused tail cols —
                regions no sign/pool ever writes."""
                Wp, H, st = pl["Wp"], pl["H"], pl["stride"]
                base = Pt[:]
                nc.gpsimd.memset(_ap(base, goff + 16, [[1, Wp]]), PAD)
                nc.gpsimd.memset(
                    _ap(base, goff + 16 + (H + 1) * Wp, [[1, Wp]]), PAD)
                used = st * B + 1
                if tail and Wp > used:
                    nc.gpsimd.memset(
                        _ap(base, goff + 16 + used, [[Wp, H + 2], [1, Wp - used]]),
                        PAD)
                nc.gpsimd.memset(_ap(base, goff, [[1, 16]]), PAD)
                tail_off = goff + 16 + (H + 2) * Wp
                ntail = pl["SZ"] - (16 + (H + 2) * Wp)
                nc.gpsimd.memset(_ap(base, tail_off, [[1, ntail]]), PAD)

            # ---- PE warm-up: burn the cold-frequency window during the
            # initial DMA wait (no data dependency; result discarded) ----
            for _ in range(10):
                psd = pp.tile([128, 512], F32, tag="ps", name="psd")
                nc.tensor.matmul(psd[:, :128], wz[:], wz[:], start=True, stop=True)

            # ---- sign emission: alternate vector(is_ge) / scalar(sigmoid) ----
            def emit_sign(dst, src, l, og, use_v):
                if use_v:
                    nc.vector.tensor_scalar(dst, src, th_t[l][:, og:og + 1], None,
                                            op0=ALU.is_ge)
                else:
                    nc.scalar.activation(dst, src, ACTF.Sigmoid,
                                         bias=sg_t[l][:, og:og + 1],
                                         scale=SIG_SCALE)


            # ---- conv1 wave 0 (image rows 1-16), 8-image octs (8KB lines) ----
            # octs ride the two HW-DGE queues (sync/scalar) only: the gpsimd
            # software queue's descriptor pump stalls whenever the gpsimd
            # engine runs memsets.
            wave_engs = [nc.scalar, nc.sync, nc.gpsimd, nc.scalar]

            def conv1_half(h, q, im):
                for j in range(8):
                    i = 8 * q + j
                    ps = pp.tile([128, 16, 32], F32, tag="ps")
                    nc.tensor.matmul(ps[:], w1_t[:], im[:, j],
                                     start=True, stop=True)
                    dst = _ap(P1[:], 16 + (1 + 16 * h) * 1072 + 33 * i + 1,
                              [[1072, 16], [1, 32]])
                    emit_sign(dst, ps[:], 1, 0, (2 * i + h) % 2 == 0)

            for q in range(B // 8):
                im = xim.tile([81, 8, 16, 32], BF16, tag="imh")
                wave_engs[q].dma_start(
                    im[:], xim_d[0, q].rearrange("k (j r w) -> k j r w", j=8, w=32))
                conv1_half(0, q, im)
            # w2 is needed by band A (~25us): put it on sync's queue
            # ahead of sync's wave-1 octs
            load_weight(2, nc.sync)
            # wave 1 DMA issues go out now (gpsimd/sync only — scalar's
            # stream is about to fill with band-A sigmoids). One buf per
            # oct: NO slot reuse (readers are emitted much later).
            w2_engs = [nc.sync, nc.gpsimd, nc.sync, nc.gpsimd]
            im2_tiles = []
            for q in range(B // 8):
                im = xim2.tile([81, 8, 16, 32], BF16, tag="imh2")
                w2_engs[q].dma_start(
                    im[:], xim_d[1, q].rearrange("k (j r w) -> k j r w", j=8, w=32))
                im2_tiles.append(im)
            load_weight(3, nc.sync)
            pad_static(P1, 0, PLANE[1])
            pad_sep(P1, 0, PLANE[1])

            # ---- dy-paired layer (IG=1): L2 (pool, banded) and L3 ----
            def mm_dy_pairs(Pin, wt, og, o, n, Wp, ps):
                k, last = 0, 4
                osl = slice(og * 128, (og + 1) * 128)
                for dx in range(3):
                    rhs = _ap(Pin[:], 16 + o - Wp + dx - 1, [[Wp, 2], [1, n]])
                    nc.tensor.matmul(ps[:], wt[:, dx, 0:2, osl], rhs,
                                     start=(k == 0), stop=(k == last),
                                     perf_mode=PM.DoubleRow)
                    k += 1
                rhs = _ap(Pin[:], 16 + o + Wp - 1, [[1, 2], [1, n]])
                nc.tensor.matmul(ps[:], wt[:, 0:2, 2, osl], rhs,
                                 start=(k == 0), stop=(k == last),
                                 perf_mode=PM.DoubleRow)
                k += 1
                rhs = _ap(Pin[:], 16 + o + Wp + 1, [[1, n]])
                nc.tensor.matmul(ps[:], wt[:, 2, 2, osl], rhs,
                                 start=(k == 0), stop=(k == last))
                k += 1

            def mm_cg_pairs(Pin, wt, og, o, n, Wp, SZg, IG, ps):
                k, last = 0, IG // 2 * 9 - 1
                for pr in range(IG // 2):
                    for dy in range(3):
                        for dx in range(3):
                            rhs = _ap(Pin[:], 2 * pr * SZg + 16 + o + (dy - 1) * Wp + dx - 1,
                                      [[SZg, 2], [1, n]])
                            nc.tensor.matmul(
                                ps[:], wt[:, 2 * pr: 2 * pr + 2, 3 * dy + dx, og, :],
                                rhs, start=(k == 0), stop=(k == last),
                                perf_mode=PM.DoubleRow)
                            k += 1

            def pool_row(scr, loc_row, Wp_in, st_in, W_half, dst_ap, tag):
                # 2x2 maxpool of scratch rows loc_row, loc_row+1 -> dst_ap
                m1 = tpool.tile([128, B, W_half], FP8, tag=f"m1{tag}")
                m2 = tpool.tile([128, B, W_half], FP8, tag=f"m2{tag}")
                for j, m in ((0, m1), (1, m2)):
                    off = (loc_row + j) * Wp_in + 1
                    nc.vector.tensor_max(
                        m[:],
                        _ap(scr[:], off, [[st_in, B], [2, W_half]]),
                        _ap(scr[:], off + 1, [[st_in, B], [2, W_half]]),
                    )
                nc.vector.tensor_max(dst_ap, m1[:], m2[:])

            # L2 in two bands: band A (out rows 1-14) only needs wave-0
            # P1 rows, so it overlaps the wave-1 DMA; band B (rows 15-32)
            # runs after conv1 wave 1.
            sk = 0  # global sign-engine alternator for conv layers

            def l2_band(row0, nrows, R0, R1):
                nonlocal sk
                scr2 = scrpool.tile([128, 18 * 1072], FP8, tag="scr")
                band0 = row0 * 1072
                total = nrows * 1072
                o = 0
                while o < total:
                    n = min(512, total - o)
                    ps = pp.tile([128, 512], F32, tag="ps")
                    mm_dy_pairs(P1, w_t[2], 0, band0 + o, n, 1072, ps[:, :n])
                    emit_sign(scr2[:, o: o + n], ps[:, :n], 2, 0, sk % 2 == 0)
                    sk += 1
                    o += n
                for R in range(R0, R1):
                    loc = (2 * R - 1) - row0
                    pool_row(scr2, loc, 1072, 33, 16,
                             _ap(P2[:], 16 + R * 560 + 1, [[17, 32], [1, 16]]), "a")

            l2_band(1, 14, 1, 8)
            # ---- conv1 wave 1 (image rows 17-32) ----
            for q in range(B // 8):
                conv1_half(1, q, im2_tiles[q])
            pad_static(P2, 0, PLANE[2])
            pad_sep(P2, 0, PLANE[2])
            for og in range(2):
                pad_static(P3, og * SZ2, PLANE[2], tail=False)
            l2_band(15, 18, 8, 17)

            load_weight(4, nc.sync)
            for og in range(2):
                pad_static(P4, og * SZ3, PLANE[3])
                pad_sep(P4, og * SZ3, PLANE[3])
            # L3. P3's separator memsets are split: rows 0-9 are re-zeroed
            # as soon as the signs covering them are done, so only a small
            # tail memset sits between og1's last sign and L4's first mm.
            def p3_sep_part(og, r0, r1):
                goff = og * SZ2
                nc.gpsimd.memset(
                    _ap(P3[:], goff + 16 + r0 * 560,
                        [[560, r1 - r0], [17, B + 1]]), PAD)
                nc.gpsimd.memset(
                    _ap(P3[:], goff + 16 + r0 * 560 + 545,
                        [[560, r1 - r0], [1, 15]]), PAD)

            for og in range(2):
                part1 = False
                for (o, n) in _pl_chunks(560, 16):
                    ps = pp.tile([128, 512], F32, tag="ps")
                    mm_dy_pairs(P2, w_t[3], og, o, n, 560, ps[:, :n])
                    emit_sign(P3[:, og, 16 + o: 16 + o + n], ps[:, :n], 3, og,
                              sk % 2 == 0)
                    sk += 1
                    if not part1 and o + n >= 10 * 560:  # rows 1..9 signed
                        p3_sep_part(og, 0, 10)
                        part1 = True
                p3_sep_part(og, 10, 18)

            load_weight(5, nc.sync)
            for og in range(4):
                pad_static(P5, og * SZ3, PLANE[3], tail=False)
            # L4 (cg pairs, pool)
            for og in range(2):
                scr4 = scrpool.tile([128, 16 * 1072], FP8, tag="scr")
                for (o, n) in _pl_chunks(560, 16):
                    ps = pp.tile([128, 512], F32, tag="ps")
                    mm_cg_pairs(P3, w_t[4], og, o, n, 560, SZ2, 2, ps[:, :n])
                    emit_sign(scr4[:, o - 560: o - 560 + n], ps[:, :n], 4, og,
                              sk % 2 == 0)
                    sk += 1
                for R in range(1, 9):
                    pool_row(scr4, 2 * (R - 1), 560, 17, 8,
                             _ap(P4[:], og * SZ3 + 16 + R * 289 + 1, [[9, 32], [1, 8]]),
                             "b")

            load_weight(6, nc.sync)
            nc.sync.dma_start(w7_t[:], w7_d[:])
            # L5
            for og in range(4):
                for (o, n) in _pl_chunks(289, 8):
                    ps = pp.tile([128, 512], F32, tag="ps")
                    mm_cg_pairs(P4, w_t[5], og, o, n, 289, SZ3, 2, ps[:, :n])
                    emit_sign(P5[:, og, 16 + o: 16 + o + n], ps[:, :n], 5, og,
                              sk % 2 == 0)
                    sk += 1
                pad_sep(P5, og * SZ3, PLANE[3], tail=True)

            # L6 (cg pairs x2, pool) with conv7 interleaved per og
            ps7 = pp7.tile([16, B], F32, tag="ps7")
            for og in range(4):
                scr6 = scrpool.tile([128, 16 * 1072], FP8, tag="scr")
                for (o, n) in _pl_chunks(289, 8):
                    ps = pp.tile([128, 512], F32, tag="ps")
                    mm_cg_pairs(P5, w_t[6], og, o, n, 289, SZ3, 4, ps[:, :n])
                    emit_sign(scr6[:, o - 289: o - 289 + n], ps[:, :n], 6, og,
                              sk % 2 == 0)
                    sk += 1
                for R in range(1, 5):
                    m1 = tpool.tile([128, B, 4], FP8, tag="m1c")
                    m2 = tpool.tile([128, B, 4], FP8, tag="m2c")
                    for j, m in ((0, m1), (1, m2)):
                        off = (2 * (R - 1) + j) * 289 + 1
                        nc.vector.tensor_max(
                            m[:],
                            _ap(scr6[:], off, [[9, B], [2, 4]]),
                            _ap(scr6[:], off + 1, [[9, B], [2, 4]]),
                        )
                    dst = _ap(buf6[:], og * 512 + (R - 1) * 128, [[32, 4], [1, B]])
                    nc.vector.tensor_max(dst,
                                         _ap(m1[:], 0, [[1, 4], [4, B]]),
                                         _ap(m2[:], 0, [[1, 4], [4, B]]))
                # conv7: stationary = w7 [128,2,16] (dy-pairs), moving = buf6
                for dx in range(4):
                    for p in range(2):
                        rhs = _ap(buf6[:], og * 512 + (2 * p) * 128 + dx * 32,
                                  [[128, 2], [1, B]])
                        nc.tensor.matmul(ps7[:], w7_t[:, og, dx, p], rhs,
                                         start=(og == 0 and dx == 0 and p == 0),
                                         stop=(og == 3 and dx == 3 and p == 1),
                                         perf_mode=PM.DoubleRow)

            # ---- copy conv7 psum out (BN + log_softmax run on the host) ----
            res = spool.tile([10, B], F32, tag="res")
            nc.scalar.copy(res[:], ps7[0:10, :])
            nc.sync.dma_start(out_d[:], res[:])

            if dump:
                for nm, bt in [("dbgP1", P1), ("dbgP2", P2), ("dbgP3", P3),
                               ("dbgP4", P4), ("dbgP5", P5), ("dbg6", buf6)]:
                    dd = nc.dram_tensor(nm, list(bt.shape), FP8, kind="ExternalOutput")
                    nc.sync.dma_start(dd[:], bt[:])

    nc.compile()
    return nc


_THOFF = {1: 0, 2: 1, 3: 2, 4: 4, 5: 6, 6: 10}


def _prep_consts(inp):
    """Host-side weight preprocessing -> (device input dict, host aux)."""
    out = {}
    ths = np.zeros((128, 28), np.float64)
    # device im2col partition order is k = dy*9 + c*3 + dx
    out["w1s"] = np.ascontiguousarray(np.tile(
        np.sign(inp["w1"]).transpose(2, 1, 3, 0).reshape(27, 128), (3, 1)
    )).astype(NP_BF16)
    for l in (4, 5, 6):
        c = CONV_CFG[l]
        IG, OG = c["IG"], c["OG"]
        ws = np.sign(inp[f"w{l}"]).astype(np.float32)  # [cout, cin, 3, 3]
        ws = ws.transpose(1, 2, 3, 0).reshape(IG, 128, 9, OG, 128)
        out[f"w{l}s"] = np.ascontiguousarray(ws.transpose(1, 0, 2, 3, 4)).astype(NP_FP8)
    for l in (2, 3):
        # dy-pair layout: [128(cin), dx, dy, cout]
        ws = np.sign(inp[f"w{l}"]).astype(np.float32)
        out[f"w{l}p"] = np.ascontiguousarray(ws.transpose(1, 3, 2, 0)).astype(NP_FP8)
    # thresholds: layer l>=2 in b-encoding: psum_b >= (W - be)/2
    for l in range(1, 7):
        g = inp[f"bn{l}_g"].astype(np.float64)
        bb = inp[f"bn{l}_b"].astype(np.float64)
        m = inp[f"bn{l}_m"].astype(np.float64)
        v = inp[f"bn{l}_v"].astype(np.float64)
        s = g / np.sqrt(v + EPS)
        t = m - bb / s
        be = inp[f"b{l}"].astype(np.float64) - t
        if l == 1:
            tp = -be
        else:
            W = np.sign(inp[f"w{l}"]).astype(np.float64).sum(axis=(1, 2, 3))
            tp = (W - be) / 2.0
        C = tp.shape[0]
        OG = C // 128
        tp = tp.reshape(OG, 128).T if OG > 1 else tp.reshape(128, 1)
        ths[:, _THOFF[l]:_THOFF[l] + tp.shape[1]] = tp
        ths[:, 14 + _THOFF[l]:14 + _THOFF[l] + tp.shape[1]] = -SIG_SCALE * tp
    # conv7: [128cin, g, dx, pair, dy-in-pair, 16(10+pad)]
    ws7 = np.sign(inp["w7"]).astype(np.float32)  # [10, 512, 4, 4]
    w7p = np.zeros((128, 4, 4, 2, 2, 16), np.float32)
    s7 = ws7.transpose(1, 2, 3, 0).reshape(4, 128, 4, 4, 10)  # [g, c, dy, dx, co]
    for p in range(2):
        for q in range(2):
            w7p[:, :, :, p, q, :10] = s7[:, :, 2 * p + q, :, :].transpose(1, 0, 2, 3)
    out["w7p"] = np.ascontiguousarray(w7p).astype(NP_FP8)
    W7 = np.sign(inp["w7"]).astype(np.float64).sum(axis=(1, 2, 3))  # [10]
    sf = inp["bnf_g"].astype(np.float64) / np.sqrt(inp["bnf_v"].astype(np.float64) + EPS)
    df = (inp["b7"].astype(np.float64) - inp["bnf_m"].astype(np.float64)) * sf + inp[
        "bnf_b"
    ].astype(np.float64)
    out["ths"] = np.ascontiguousarray(ths).astype(np.float32)
    aux = dict(W7=W7, sf=sf, df=df)
    return out, aux


def _prep_x_im2col(x):
    """[b,3,32,32] -> [b,27,2,1024] zero-padded im2col (bf16 hi/lo split),
    k = dy*9 + c*3 + dx."""
    b = x.shape[0]
    xp = np.zeros((b, 3, 34, 34), np.float32)
    xp[:, :, 1:33, 1:33] = x
    xim = np.empty((b, 27, 32, 32), np.float32)
    for dy in range(3):
        for c in range(3):
            for dx in range(3):
                xim[:, dy * 9 + c * 3 + dx] = xp[:, c, dy: dy + 32, dx: dx + 32]
    xim = xim.reshape(b, 27, 1024)
    t0 = xim.astype(NP_BF16)
    r1 = xim - t0.astype(np.float32)
    t1 = r1.astype(NP_BF16)
    t2 = (r1 - t1.astype(np.float32)).astype(NP_BF16)
    full = np.concatenate([t0, t1, t2], axis=1)          # [b, 81, 1024]
    # [half, oct, 81, (8 img, 16 rows, 32 cols)]
    v = full.reshape(b // 8, 8, 81, 2, 16 * 32)
    v = v.transpose(3, 0, 2, 1, 4)                       # [2, o, 81, 8, 512]
    return np.ascontiguousarray(v.reshape(2, b // 8, 81, 4096))


def make_in_maps(inputs):
    consts, aux = _prep_consts(inputs)
    x = np.asarray(inputs["x"], dtype=np.float32)
    in_maps = []
    for c in range(N_CORES):
        m = dict(consts)
        m["xim"] = _prep_x_im2col(x[c * B: (c + 1) * B])
        in_maps.append(m)
    return in_maps, aux


def finish_host(res_list, aux):
    """[10,B] per core psum_b -> full [256,10] log_softmax output."""
    psb = np.concatenate([np.asarray(r["out"], np.float64) for r in res_list],
                         axis=1)  # [10, 256]
    true = 2.0 * psb - aux["W7"][:, None]
    z = (true * aux["sf"][:, None] + aux["df"][:, None]).T  # [256, 10]
    zm = z - z.max(axis=1, keepdims=True)
    out = zm - np.log(np.exp(zm).sum(axis=1, keepdims=True))
    return out.astype(np.float32)


def kernel(**inputs) -> np.ndarray:
    inputs = {k: np.asarray(v) for k, v in inputs.items()}
    if "nc" not in _CACHE:
        _CACHE["nc"] = _build_v3()
    nc = _CACHE["nc"]
    in_maps, aux = make_in_maps(inputs)
    res = run_bass_kernel_spmd(nc, in_maps, list(range(N_CORES)))
    return finish_host(res.results, aux)
